# revision 19
# baseline (speedup 1.0000x reference)
"""Fastmax (p=1 causal linear attention) Trainium2 kernel, 8-core SPMD.

Sharding: data-parallel over heads (16 heads -> 2 per core). Each core
computes q/k/v projections for its 2 heads, the chunked causal linear
attention (augmented [65,65] prefix state carries S, ksum, vsum, count),
and a partial output projection. Host sums the 8 partial outputs + bias.

v2 design notes (cost-model driven):
  - xt streamed as 8 big [128, N] DMAs; phase A (q,k) accumulates
    k-outer into 7 resident psum banks so PE consumption paces DMA
    arrival. One k psum (n0=3) is deferred and rotated into a freed
    bank so a warmup bank exists (p-state ramp: idle PE restarts slow).
  - normalization trick: with the q aug row holding c = qn*kn instead
    of 1, every score scales uniformly by 1/s and num/den is invariant,
    so q/k are never rescaled; c lands in the aug row via a tiny
    PE broadcast once the norm stats resolve.
  - norm stats: 8 [2,512] per-token-norm matmuls stack into one
    [16,512] psum -> single DVE max-reduce -> PE transpose -> tiny
    strided reduces; keeps the c critical path ~1us.
  - krows (token-major k for the state sweep) come from PE transposes
    of kh instead of a second k projection.
  - causal state is chunk-granular (128 tokens): no unmasked score
    block; per chunk O = masked-diag + q @ S_prev.
  - single-shot matmul outputs (scores, outproj, state deltas) land in
    bf16 psum: the psum->sbuf drain gets DVE 2x mode and bf16 volume.
  - copies merge both heads into one instruction via nested-stride APs.
"""

import sys

sys.path.insert(0, "/opt/trn_rl_repo")

import numpy as np

B, N, D_MODEL, H, D_HEAD = 1, 2048, 1024, 16, 64
NCORES = 8
HPC = H // NCORES  # heads per core
DPC = HPC * D_HEAD  # out dims per core (128)
CH = 128  # chunk (tokens)
SPAN = 256  # query span (2 chunks)
NSPAN = N // SPAN
KT = D_MODEL // 128  # contraction tiles for projections
NCH = N // CH  # token chunks (16)
RST = 80  # row-buffer stride per chunk (64 data + ones col + pad)
HRST = NCH * RST  # per-head stride inside krows/vrows
NWARM = 13  # PE warmup matmuls during initial DMA wait

_CACHE = {}


def _build():
    import concourse.bass as bass
    import concourse.tile as tile
    import concourse.mybir as mybir
    from concourse import bacc
    from concourse.alu_op_type import AluOpType

    BF = mybir.dt.bfloat16
    F32 = mybir.dt.float32
    AF = mybir.ActivationFunctionType
    AX = mybir.AxisListType

    nc = bacc.Bacc("TRN2", target_bir_lowering=False, debug=False, num_devices=NCORES)

    xt_d = nc.declare_dram_parameter("xt", [D_MODEL, N], BF, isOutput=False)
    wqk_d = nc.declare_dram_parameter("wqk", [128, 2 * D_MODEL], BF, isOutput=False)
    wvo_d = nc.declare_dram_parameter("wvo", [128, 2 * D_MODEL], BF, isOutput=False)
    consts_d = nc.declare_dram_parameter("consts", [128, 577], BF, isOutput=False)
    onesr_d = nc.declare_dram_parameter("onesr", [1, HPC * 2 * N], BF, isOutput=False)
    out_d = nc.declare_dram_parameter("out", [N, D_MODEL], BF, isOutput=True)

    def ap3(t, off, free_dims, nparts=128):
        # raw AP: partition dim step is the tile's row pitch (elements)
        pitch = t.ap[0][0]
        return bass.AP(t.tensor, t.offset + off, [[pitch, nparts]] + free_dims)

    with tile.TileContext(nc) as tc:
        with (
            tc.tile_pool(name="const", bufs=1) as constp,
            tc.tile_pool(name="wts", bufs=1) as wp,
            tc.tile_pool(name="acts", bufs=1) as actp,
            tc.tile_pool(name="mt", bufs=4) as mtp,
            tc.tile_pool(name="ssb", bufs=1) as ssbp,
            tc.tile_pool(name="recp", bufs=8) as recp,
            tc.tile_pool(name="vhrp", bufs=4) as vhrp,
            tc.tile_pool(name="osb", bufs=3) as osbp,
            tc.tile_pool(name="nf", bufs=1) as nf,
        ):
            consts = constp.tile([128, 577], BF)
            nc.sync.dma_start(consts[:], consts_d[:])
            ident = consts[:, 0:128]
            dmask = consts[:, 128:384]
            e64 = consts[0:1, 512:577]

            # qkh column blocks: [q0 | q1 | k0 | k1], each [65, N]; row 64 is
            # the aug row (ones for k, c=qn*kn for q).
            qkh = actp.tile([65, HPC * 2 * N], BF, tag="qkh")
            nc.sync.dma_start(qkh[64:65, :], onesr_d[:])

            wqk_sb = wp.tile([128, 2 * D_MODEL], BF)
            nc.sync.dma_start(wqk_sb[:], wqk_d[:])

            xts = []
            for k in range(KT):
                xtile = actp.tile([128, N], BF, tag=f"xt{k}", name=f"xt{k}")
                nc.sync.dma_start(xtile[:], xt_d[k * 128 : (k + 1) * 128, :])
                xts.append(xtile)

            wvo_sb = wp.tile([128, 2 * D_MODEL], BF)
            nc.sync.dma_start(wvo_sb[:], wvo_d[:])

            vht = actp.tile([128, N], BF, tag="vht")
            krows = actp.tile([128, HPC * HRST], BF, tag="krows")
            vrows = actp.tile([128, HPC * HRST], BF, tag="vrows")
            # aug columns (64 mod RST) of vrows must be ones; data cols are
            # overwritten by the v-projection copies later
            nc.gpsimd.memset(vrows[:], 1.0)

            def qb(h):
                return h * N

            def kb(h):
                return (HPC + h) * N

            sqp = {}

            # ================= warmup + phase A: q,k projections =================
            with tc.tile_pool(name="warm", bufs=1, space="PSUM") as warmp:
                wps = warmp.tile([128, 386], F32)
                for i in range(NWARM):
                    nc.tensor.matmul(wps[:], ident, consts[:, 0:386], start=True, stop=True)

            with tc.tile_pool(name="projps", bufs=1, space="PSUM") as pps:
                pq = [pps.tile([128, 512], F32, tag=f"pq{n0}", name=f"pq{n0}") for n0 in range(4)]
                pk = [pps.tile([128, 512], F32, tag=f"pk{n0}", name=f"pk{n0}") for n0 in range(3)]

                def drain(p, name, blk0, n0):
                    j = drain.cnt
                    drain.cnt += 1
                    cs0, cs1 = n0 * 512, (n0 + 1) * 512
                    sq = actp.tile([128, 512], BF, tag=f"sq{name}", name=f"sq{name}")
                    nc.scalar.activation(sq[:], p[:], AF.Square)
                    sqp[name] = sq
                    for h in range(HPC):
                        dst = qkh[0:64, (blk0 + h) * N + cs0 : (blk0 + h) * N + cs1]
                        src = p[h * 64 : (h + 1) * 64, :]
                        if (j + h) % 2 == 0:
                            nc.vector.tensor_copy(dst, src)
                        else:
                            nc.scalar.copy(dst, src)

                drain.cnt = 0

                for k in range(KT):
                    ws = wqk_sb[:, k * 128 : (k + 1) * 128]
                    wsk = wqk_sb[:, D_MODEL + k * 128 : D_MODEL + (k + 1) * 128]
                    for n0 in range(4):
                        nc.tensor.matmul(
                            pq[n0][:],
                            ws,
                            xts[k][:, n0 * 512 : (n0 + 1) * 512],
                            start=(k == 0),
                            stop=(k == KT - 1),
                        )
                    if k == KT - 1:
                        for n0 in range(4):
                            drain(pq[n0], f"q{n0}", 0, n0)
                    for n0 in range(3):
                        nc.tensor.matmul(
                            pk[n0][:],
                            wsk,
                            xts[k][:, n0 * 512 : (n0 + 1) * 512],
                            start=(k == 0),
                            stop=(k == KT - 1),
                        )
                    if k == KT - 1:
                        for n0 in range(3):
                            drain(pk[n0], f"k{n0}", HPC, n0)

                # deferred k n0=3 rotates into pq0's bank
                pk3 = pps.tile([128, 512], F32, tag="pq0", name="pk3")
                for k in range(KT):
                    nc.tensor.matmul(
                        pk3[:],
                        wqk_sb[:, D_MODEL + k * 128 : D_MODEL + (k + 1) * 128],
                        xts[k][:, 3 * 512 : 4 * 512],
                        start=(k == 0),
                        stop=(k == KT - 1),
                    )
                drain(pk3, "k3", HPC, 3)

            # ============ region 2: norms, v, k-transposes, attention ============
            with (
                tc.tile_pool(name="nrmps", bufs=1, space="PSUM") as nps,
                tc.tile_pool(name="vkps", bufs=2, space="PSUM") as vkps,
                tc.tile_pool(name="dlps", bufs=1, space="PSUM") as dlps,
                tc.tile_pool(name="bigps", bufs=2, space="PSUM") as bigps,
                tc.tile_pool(name="oops", bufs=2, space="PSUM") as oops,
            ):
                # --- norm stats: 8 matmuls stack into one [16,512] psum ---
                nrm16 = nps.tile([16, 512], F32, tag="nrm", name="nrm16")
                for j, name in enumerate(
                    [f"q{i}" for i in range(4)] + [f"k{i}" for i in range(4)]
                ):
                    nc.tensor.matmul(
                        nrm16[:],
                        consts[:, 384 + 16 * j : 400 + 16 * j],
                        sqp[name][:],
                        start=(j == 0),
                        stop=(j == 7),
                    )
                nr16 = nf.tile([16, 1], BF)
                nc.vector.tensor_reduce(nr16[:], nrm16[:], AX.X, AluOpType.max)
                tr16 = nps.tile([1, 16], BF, tag="nrm", name="tr16")

                s_chain = {}
                s_snap = {}
                csc = {}

                def vktr_chunk(ci):
                    ts0 = ci * 128
                    pv = vkps.tile([128, 128], F32, tag="vk", name="pv")
                    for k in range(KT):
                        nc.tensor.matmul(
                            pv[:],
                            xts[k][:, ts0 : ts0 + 128],
                            wvo_sb[:, k * 128 : (k + 1) * 128],
                            start=(k == 0),
                            stop=(k == KT - 1),
                        )
                    dst = ap3(vrows[:], ci * RST, [[HRST, HPC], [1, 64]])
                    src = ap3(pv[:], 0, [[64, HPC], [1, 64]])
                    if ci % 2 == 0:
                        nc.vector.tensor_copy(dst, src)
                    else:
                        nc.scalar.copy(dst, src)
                    ktp = vkps.tile([128, 132], BF, tag="vk", name="ktp")
                    for h in range(HPC):
                        nc.tensor.transpose(
                            ktp[:, h * 66 : h * 66 + 65],
                            qkh[0:65, kb(h) + ts0 : kb(h) + ts0 + 128],
                            ident[0:65, 0:65],
                        )
                    dst = ap3(krows[:], ci * RST, [[HRST, HPC], [1, 65]])
                    src = ap3(ktp[:], 0, [[66, HPC], [1, 65]])
                    if ci % 2 == 0:
                        nc.scalar.copy(dst, src)
                    else:
                        nc.vector.tensor_copy(dst, src)

                def sweep_chunk(ci):
                    dl = dlps.tile([65, 2 * 65], F32, tag="dl", name="dl")
                    for h in range(HPC):
                        nc.tensor.matmul(
                            dl[:, h * 65 : (h + 1) * 65],
                            krows[:, h * HRST + ci * RST : h * HRST + ci * RST + 65],
                            vrows[:, h * HRST + ci * RST : h * HRST + ci * RST + 65],
                            start=True,
                            stop=True,
                            skip_group_check=True,
                        )
                    for h in range(HPC):
                        ch = ssbp.tile(
                            [65, 65], F32, tag=f"sch{h}_{ci}", name=f"sch{h}_{ci}", bufs=1
                        )
                        if ci == 0:
                            nc.vector.tensor_copy(ch[:], dl[:, h * 65 : (h + 1) * 65])
                        else:
                            nc.vector.tensor_add(
                                ch[:], dl[:, h * 65 : (h + 1) * 65], s_chain[(h, ci - 1)][:]
                            )
                        s_chain[(h, ci)] = ch
                        s_sb = ssbp.tile(
                            [65, 65], BF, tag=f"ssb{h}_{ci}", name=f"ssb{h}_{ci}", bufs=1
                        )
                        nc.gpsimd.tensor_copy(s_sb[:], ch[:])
                        s_snap[(h, ci)] = s_sb

                def c_finalize():
                    # tr16 column j holds mm j's (h0,h1) maxima pair; q cols
                    # {h,2+h,..}, k cols {8+h,..}. tiny strided reduces -> c_h.
                    for h in range(HPC):
                        mqh = nf.tile([1, 1], F32, tag=f"mq{h}", name=f"mq{h}")
                        mkh = nf.tile([1, 1], F32, tag=f"mk{h}", name=f"mk{h}")
                        nc.vector.tensor_reduce(
                            mqh[:], ap3(tr16[:], h, [[2, 4]], nparts=1), AX.X, AluOpType.max
                        )
                        nc.vector.tensor_reduce(
                            mkh[:], ap3(tr16[:], 8 + h, [[2, 4]], nparts=1), AX.X, AluOpType.max
                        )
                        pr = nf.tile([1, 1], F32, tag=f"pr{h}", name=f"pr{h}")
                        nc.vector.tensor_mul(pr[:], mqh[:], mkh[:])
                        rt = nf.tile([1, 1], F32, tag=f"rt{h}", name=f"rt{h}")
                        nc.scalar.activation(rt[:], pr[:], AF.Sqrt)
                        rtb = nf.tile([1, 1], BF, tag=f"rtb{h}", name=f"rtb{h}")
                        nc.vector.tensor_copy(rtb[:], rt[:])
                        cb65 = nps.tile([65, 1], F32, tag="nrm", name=f"cb65_{h}")
                        nc.tensor.matmul(cb65[:], e64, rtb[:], start=True, stop=True)
                        cs_ = nf.tile([65, 1], F32, tag=f"csc{h}", name=f"csc{h}")
                        nc.vector.tensor_copy(cs_[:], cb65[:])
                        csc[h] = cs_
                    for h in range(HPC):  # span 0 cols first: unblock scores(0)
                        row = qkh[64:65, qb(h) : qb(h) + SPAN]
                        nc.vector.tensor_scalar_mul(row, row, csc[h][64:65, :])
                    for h in range(HPC):
                        row = qkh[64:65, qb(h) + SPAN : qb(h) + N]
                        nc.vector.tensor_scalar_mul(row, row, csc[h][64:65, :])

                def scores(sp):
                    mtds = []
                    for h in range(HPC):
                        ptj = bigps.tile([128, SPAN], F32, tag="big", name="ptj")
                        for i, ci in enumerate((2 * sp, 2 * sp + 1)):
                            nc.tensor.matmul(
                                ptj[:, i * CH : (i + 1) * CH],
                                qkh[0:65, kb(h) + ci * CH : kb(h) + (ci + 1) * CH],
                                qkh[0:65, qb(h) + ci * CH : qb(h) + (ci + 1) * CH],
                                start=True,
                                stop=True,
                                skip_group_check=True,
                            )
                        mtd = mtp.tile([128, SPAN], BF, tag="mtd", name="mtd")
                        nc.vector.tensor_mul(mtd[:], ptj[:], dmask)
                        mtds.append(mtd)
                    return mtds

                def o_part(sp, mtds):
                    ca, cb_ = 2 * sp, 2 * sp + 1
                    vhrs = {
                        ca: vhrp.tile([128, 128], BF, tag="vhr", name="vhra"),
                        cb_: vhrp.tile([128, 128], BF, tag="vhr", name="vhrb"),
                    }
                    for h in range(HPC):
                        for i, ci in enumerate((ca, cb_)):
                            o = oops.tile([128, 65], F32, tag="oop", name="o")
                            vr = vrows[:, h * HRST + ci * RST : h * HRST + ci * RST + 65]
                            nc.tensor.matmul(
                                o[:],
                                mtds[h][:, i * CH : (i + 1) * CH],
                                vr,
                                start=True,
                                stop=(ci == 0),
                            )
                            if ci > 0:
                                nc.tensor.matmul(
                                    o[:],
                                    qkh[0:65, qb(h) + ci * CH : qb(h) + (ci + 1) * CH],
                                    s_snap[(h, ci - 1)][:],
                                    start=False,
                                    stop=True,
                                )
                            rec = recp.tile([128, 1], F32, tag="rec", name="rec")
                            nc.vector.reciprocal(rec[:], o[:, 64:65])
                            dst = vhrs[ci][:, h * 64 : (h + 1) * 64]
                            if (h + i) % 2 == 0:
                                nc.vector.tensor_scalar_mul(dst, o[:, 0:64], rec[:])
                            else:
                                nc.scalar.activation(dst, o[:, 0:64], AF.Copy, scale=rec[:])
                    return vhrs

                def vht_finish(sp, vhrs):
                    vtp = bigps.tile([128, SPAN], BF, tag="big", name="vtp")
                    for i, ci in enumerate((2 * sp, 2 * sp + 1)):
                        nc.tensor.transpose(vtp[:, i * CH : (i + 1) * CH], vhrs[ci][:], ident)
                    nc.scalar.copy(vht[:, sp * SPAN : (sp + 1) * SPAN], vtp[:])

                def outproj_mm(sp):
                    for r in (2 * sp, 2 * sp + 1):
                        rs_ = slice(r * CH, (r + 1) * CH)
                        ob = osbp.tile([128, D_MODEL], BF, tag="osb", name="osb")
                        for n2 in range(D_MODEL // 512):
                            ns = slice(n2 * 512, (n2 + 1) * 512)
                            op = oops.tile([128, 512], F32, tag="oop", name="opps")
                            nc.tensor.matmul(
                                op[:],
                                vht[:, rs_],
                                wvo_sb[:, D_MODEL + ns.start : D_MODEL + ns.stop],
                                start=True,
                                stop=True,
                            )
                            if (r + n2) % 2 == 0:
                                nc.vector.tensor_copy(ob[:, ns], op[:])
                            else:
                                nc.scalar.copy(ob[:, ns], op[:])
                        nc.sync.dma_start(out_d[rs_, :], ob[:])

                vhr_prev = None
                for sp in range(NSPAN):
                    ca, cb_ = 2 * sp, 2 * sp + 1
                    vktr_chunk(ca)
                    sweep_chunk(ca)
                    if sp == 0:
                        nc.tensor.transpose(tr16[:], nr16[:], ident[0:16, 0:16])
                    else:
                        vht_finish(sp - 1, vhr_prev)
                    vktr_chunk(cb_)
                    if cb_ < NCH - 1:
                        sweep_chunk(cb_)
                    if sp == 0:
                        c_finalize()
                    mtds = scores(sp)
                    if sp >= 1:
                        outproj_mm(sp - 1)
                    vhr_prev = o_part(sp, mtds)
                vht_finish(NSPAN - 1, vhr_prev)
                outproj_mm(NSPAN - 1)

    nc.compile()
    return nc


def _consts():
    import ml_dtypes

    bf = ml_dtypes.bfloat16
    consts = np.zeros((128, 577), dtype=np.float32)
    consts[:, 0:128] = np.eye(128)
    j = np.arange(128)[:, None]
    i = np.arange(CH)[None, :]
    tri = (j <= i).astype(np.float32)
    consts[:, 128:256] = tri
    consts[:, 256:384] = tri
    # hindt16 blocks: mm j's stationary [128,16] has only cols 2j (head0
    # rows) and 2j+1 (head1 rows) set, so 8 accumulating matmuls stack
    # per-(proj,n0) norm rows into one [16,512] psum.
    for jj in range(8):
        for h in range(HPC):
            consts[h * 64 : (h + 1) * 64, 384 + 16 * jj + 2 * jj + h] = 1.0
    consts[0, 576] = 1.0  # e64: [1,65] one-hot at col 64 (cols 512..576)
    onesr = np.ones((1, HPC * 2 * N), dtype=bf)
    return consts.astype(bf), onesr


def _in_maps(inputs):
    import ml_dtypes

    bf = ml_dtypes.bfloat16
    X = np.ascontiguousarray(np.asarray(inputs["X"], dtype=np.float32))
    xt = np.ascontiguousarray(X[0].T).astype(bf)  # [D_MODEL, N]
    wqt = np.ascontiguousarray(np.asarray(inputs["Wq"], np.float32).T).astype(bf)
    wkt = np.ascontiguousarray(np.asarray(inputs["Wk"], np.float32).T).astype(bf)
    wvt = np.ascontiguousarray(np.asarray(inputs["Wv"], np.float32).T).astype(bf)
    wot = np.ascontiguousarray(np.asarray(inputs["Wo"], np.float32).T).astype(bf)
    consts, onesr = _consts()

    def sb_layout(w):  # [1024, 128] -> [128, 8*128] (dm-chunk on partitions)
        return np.ascontiguousarray(
            w.reshape(KT, 128, DPC).transpose(1, 0, 2).reshape(128, KT * DPC)
        )

    in_maps = []
    for c in range(NCORES):
        cs = slice(c * DPC, (c + 1) * DPC)
        wqk = np.concatenate([sb_layout(wqt[:, cs]), sb_layout(wkt[:, cs])], axis=1)
        wvo = np.concatenate(
            [sb_layout(wvt[:, cs]), np.ascontiguousarray(wot[cs, :])], axis=1
        )
        in_maps.append(
            {
                "xt": xt,
                "wqk": np.ascontiguousarray(wqk),
                "wvo": np.ascontiguousarray(wvo),
                "consts": consts,
                "onesr": onesr,
            }
        )
    return in_maps


def _run(inputs, trace=False):
    from concourse.bass_utils import run_bass_kernel_spmd

    if "nc" not in _CACHE:
        _CACHE["nc"] = _build()
    nc = _CACHE["nc"]
    in_maps = _in_maps(inputs)
    res = run_bass_kernel_spmd(nc, in_maps, core_ids=list(range(NCORES)), trace=trace)
    bo = np.asarray(inputs["bo"], dtype=np.float32)
    acc = np.zeros((N, D_MODEL), dtype=np.float32)
    for c in range(NCORES):
        acc += res.results[c]["out"].astype(np.float32)
    acc += bo[None, :]
    return acc.reshape(B, N, D_MODEL), res.exec_time_ns


def kernel(**inputs) -> np.ndarray:
    out, _ = _run(inputs, trace=False)
    return out
